# revision 10
# baseline (speedup 1.0000x reference)
"""Trainium2 Bass kernel for nn_EncoderTransformer (ragged bag-of-words ConcatAttention).

Sharding: data-parallel over the bag dim T = B*C*N = 8192 -> 1024 bags/core on 8 cores
(= 8 samples/core, since each sample b owns C*N = 128 bags).

Per-core pipeline (per 128-token tile = 4 bags, all within one sample):
  1. indirect-DMA gather of emb rows with f32->bf16 cast; word-masked tokens have
     index 0 (PAD row, all zeros) so word masking is free
  2. PE transposes of the 4 [128tok,128d] chunks (bf16) -> bagsT in [d, tok] layout
  3. pre[tok,k] = sum_ch bagsT_ch.T @ W_pre_ch  (+ q-row via K=1 ones matmul), PSUM f32
  4. energy = v . tanh(pre) via ACT tanh + DVE mul/reduce; exp on ACT (no max-sub:
     |energy| <= ||v||_1, safely in fp32 exp range)
  5. unnormalized scores written as a block-diagonal "comb" [128tok, 4bag]
  6. contextT[d, bag] += bags_ch.T @ comb  (contraction over tokens), PSUM [128, 512]
     accumulated across the 32 tiles of a bagtile (128 bags)
  7. per-bagtile: sumexp via comb_all matmul with ones -> [128bag, 1]; context
     transposed back on PE; final DVE scale by (1/sumexp) * output-mask; DMA out
"""
import numpy as np
from contextlib import ExitStack

import concourse.bass as bass
import concourse.tile as tile
from concourse import bacc, mybir
from concourse.bass_utils import run_bass_kernel_spmd
from concourse.masks import make_identity

# problem constants (hardcoded per contract)
B, C, N, W = 64, 4, 32, 32
D, DK, V = 512, 64, 32000
NCORES = 8
SPC = B // NCORES          # samples per core = 8
BAGS = SPC * C * N         # bags per core = 1024
NT = BAGS // 4             # 128-token tiles per core = 256
NBT = BAGS // 128          # bagtiles per core = 8

f32 = mybir.dt.float32
bf16 = mybir.dt.bfloat16
i32 = mybir.dt.int32

P = 128


def _emit(nc, tc, ctx, t):
    ids, nlen, hid, emb, wpre, wq, bpre, vvec, nsz, clen, out = (
        t["ids"], t["nlen"], t["hid"], t["emb"], t["wpre"], t["wq"],
        t["bpre"], t["vvec"], t["nsz"], t["clen"], t["out"])

    cp = ctx.enter_context(tc.tile_pool(name="const", bufs=1))
    pp = ctx.enter_context(tc.tile_pool(name="prep", bufs=1))
    gp = ctx.enter_context(tc.tile_pool(name="gather", bufs=6))
    btp = ctx.enter_context(tc.tile_pool(name="bT", bufs=8))
    ep = ctx.enter_context(tc.tile_pool(name="energy", bufs=3))
    combp = ctx.enter_context(tc.tile_pool(name="comb", bufs=2))
    op = ctx.enter_context(tc.tile_pool(name="outp", bufs=2))
    tp_ps = ctx.enter_context(tc.tile_pool(name="tp", bufs=2, space="PSUM"))
    pre_ps = ctx.enter_context(tc.tile_pool(name="pre", bufs=2, space="PSUM"))
    ctx_ps = ctx.enter_context(tc.tile_pool(name="ctx", bufs=2, space="PSUM"))
    sum_ps = ctx.enter_context(tc.tile_pool(name="sums", bufs=2, space="PSUM"))

    # ---------------- statics ----------------
    id_bf = cp.tile([P, P], bf16, tag="id_bf")
    make_identity(nc, id_bf[:])
    id_f = cp.tile([P, P], f32, tag="id_f")
    make_identity(nc, id_f[:])

    # blockmask[p, c] = 1.0 if p//32 == c else 0  ([128, 4])
    bm = cp.tile([P, 4], f32, tag="bm")
    nc.gpsimd.memset(bm[:], 1.0)
    nc.gpsimd.affine_select(out=bm[:], in_=bm[:], compare_op=mybir.AluOpType.is_ge,
                            fill=0.0, base=0, pattern=[[-32, 4]], channel_multiplier=1)
    nc.gpsimd.affine_select(out=bm[:], in_=bm[:], compare_op=mybir.AluOpType.is_ge,
                            fill=0.0, base=31, pattern=[[32, 4]], channel_multiplier=-1)
    # blockmaskT[c, p] ([4, 128])
    bmT = cp.tile([4, P], f32, tag="bmT")
    nc.gpsimd.memset(bmT[:], 1.0)
    nc.gpsimd.affine_select(out=bmT[:], in_=bmT[:], compare_op=mybir.AluOpType.is_ge,
                            fill=0.0, base=0, pattern=[[1, P]], channel_multiplier=-32)
    nc.gpsimd.affine_select(out=bmT[:], in_=bmT[:], compare_op=mybir.AluOpType.is_ge,
                            fill=0.0, base=31, pattern=[[-1, P]], channel_multiplier=32)

    ones_bf = cp.tile([1, P], bf16, tag="ones_bf")
    nc.vector.memset(ones_bf[:], 1.0)
    onescol_bf = cp.tile([P, 1], bf16, tag="onescol_bf")
    nc.vector.memset(onescol_bf[:], 1.0)
    ones_f = cp.tile([1, P], f32, tag="ones_f")
    nc.vector.memset(ones_f[:], 1.0)

    # ---------------- weights ----------------
    wpre_f = cp.tile([P, 4 * DK], f32, tag="wpre_f")
    nc.sync.dma_start(wpre_f[:].rearrange("p (c k) -> p c k", c=4),
                      wpre[:].rearrange("(c p) k -> p c k", p=P))
    wpre_b = cp.tile([P, 4 * DK], bf16, tag="wpre_b")
    nc.vector.tensor_copy(wpre_b[:], wpre_f[:])
    wq_f = cp.tile([P, 4 * DK], f32, tag="wq_f")
    nc.sync.dma_start(wq_f[:].rearrange("p (c k) -> p c k", c=4),
                      wq[:].rearrange("(c p) k -> p c k", p=P))
    bpre_sb = cp.tile([1, DK], f32, tag="bpre_sb")
    nc.sync.dma_start(bpre_sb[:], bpre[:].rearrange("(o k) -> o k", o=1))
    v_row = cp.tile([1, DK], f32, tag="v_row")
    nc.sync.dma_start(v_row[:], vvec[:].rearrange("(o k) -> o k", o=1))

    # v_tile [128, 64] = broadcast of v across partitions (K=1 matmul)
    vt_psum = pre_ps.tile([P, DK], f32, tag="pre")
    nc.tensor.matmul(out=vt_psum[:], lhsT=ones_f[:], rhs=v_row[:], start=True, stop=True)
    v_tile = cp.tile([P, DK], f32, tag="v_tile")
    nc.vector.tensor_copy(v_tile[:], vt_psum[:])

    # ---------------- masked gather indices ----------------
    ids_nat = pp.tile([P, 2 * P], i32, tag="ids_nat")
    nc.sync.dma_start(ids_nat[:].rearrange("p (x w) -> p x w", x=8),
                      ids[:].rearrange("(p x) w -> p x w", p=P))
    nl_nat = pp.tile([P, 8], i32, tag="nl_nat")
    nc.sync.dma_start(nl_nat[:], nlen[:].rearrange("(p j) -> p j", p=P))
    iota_w = pp.tile([P, 2 * P], i32, tag="iota_w")
    nc.gpsimd.iota(iota_w[:].rearrange("p (x w) -> p x w", x=8),
                   pattern=[[0, 8], [1, 32]], base=0, channel_multiplier=0)
    maskw = pp.tile([P, 2 * P], i32, tag="maskw")
    nc.vector.tensor_tensor(
        out=maskw[:].rearrange("p (x w) -> p x w", x=8),
        in0=iota_w[:].rearrange("p (x w) -> p x w", x=8),
        in1=nl_nat[:].rearrange("p (x o) -> p x o", o=1).to_broadcast([P, 8, 32]),
        op=mybir.AluOpType.is_lt)
    idm_i = pp.tile([P, 2 * P], i32, tag="idm_i")
    nc.vector.tensor_mul(idm_i[:], ids_nat[:], maskw[:])
    idm_f = pp.tile([P, 2 * P], f32, tag="idm_f")
    nc.vector.tensor_copy(idm_f[:], idm_i[:])
    # transpose halves: idx col layout -> tile t at column 128*(t%2) + t//2
    idx_i = cp.tile([P, 2 * P], i32, tag="idx_i")
    for h in range(2):
        t_ps = tp_ps.tile([P, P], f32, tag="tp")
        nc.tensor.transpose(out=t_ps[:], in_=idm_f[:, P * h:P * (h + 1)], identity=id_f[:])
        nc.vector.tensor_copy(idx_i[:, P * h:P * (h + 1)], t_ps[:])

    # ---------------- q per sample ----------------
    h_nat = pp.tile([SPC, D], f32, tag="h_nat")
    nc.sync.dma_start(h_nat[:], hid[:])
    hT = pp.tile([P, 4 * SPC], f32, tag="hT")
    for ch in range(4):
        t_ps = tp_ps.tile([P, SPC], f32, tag="tp")
        nc.tensor.transpose(out=t_ps[:], in_=h_nat[:, P * ch:P * (ch + 1)],
                            identity=id_f[:SPC, :SPC])
        nc.vector.tensor_copy(hT[:, SPC * ch:SPC * (ch + 1)], t_ps[:])
    q_psum = pre_ps.tile([SPC, DK], f32, tag="pre")
    for ch in range(4):
        nc.tensor.matmul(out=q_psum[:], lhsT=hT[:, SPC * ch:SPC * (ch + 1)],
                         rhs=wq_f[:, DK * ch:DK * (ch + 1)],
                         start=(ch == 0), stop=False)
    nc.tensor.matmul(out=q_psum[:], lhsT=ones_f[:, :SPC], rhs=bpre_sb[:],
                     start=False, stop=True)
    q_sb = pp.tile([SPC, DK], f32, tag="q_sb")
    nc.vector.tensor_copy(q_sb[:], q_psum[:])
    # flatten to one partition so per-sample rows slice at base partition 0
    q_lin = pp.tile([1, SPC * DK], f32, tag="q_lin")
    nc.sync.dma_start(q_lin[:].rearrange("o (s k) -> o s k", s=SPC), q_sb[:])
    q_bf = cp.tile([1, SPC * DK], bf16, tag="q_bf")
    nc.vector.tensor_copy(q_bf[:], q_lin[:])

    # ---------------- output bag mask ----------------
    ns_i = pp.tile([4, SPC], i32, tag="ns_i")
    nc.sync.dma_start(ns_i[:], nsz[:].rearrange("(bt c) -> c bt", c=4))
    ns_f = pp.tile([4, SPC], f32, tag="ns_f")
    nc.vector.tensor_copy(ns_f[:], ns_i[:])
    cl_i = pp.tile([1, SPC], i32, tag="cl_i")
    nc.sync.dma_start(cl_i[:], clen[:].rearrange("(o j) -> o j", o=1))
    cl_f = pp.tile([1, SPC], f32, tag="cl_f")
    nc.vector.tensor_copy(cl_f[:], cl_i[:])
    ns_exp = pre_ps.tile([P, SPC], f32, tag="pre")
    nc.tensor.matmul(out=ns_exp[:], lhsT=bmT[:], rhs=ns_f[:], start=True, stop=True)
    cl_exp = sum_ps.tile([P, SPC], f32, tag="sums")
    nc.tensor.matmul(out=cl_exp[:], lhsT=ones_f[:], rhs=cl_f[:], start=True, stop=True)
    # n_col[p] = p % 32, c_col[p] = p // 32 via free-dim iota + PE transpose
    iota_nf = pp.tile([1, P], i32, tag="iota_nf")
    nc.gpsimd.iota(iota_nf[:].rearrange("p (c w) -> p c w", c=4),
                   pattern=[[0, 4], [1, 32]], base=0, channel_multiplier=0)
    iota_cf = pp.tile([1, P], i32, tag="iota_cf")
    nc.gpsimd.iota(iota_cf[:].rearrange("p (c w) -> p c w", c=4),
                   pattern=[[1, 4], [0, 32]], base=0, channel_multiplier=0)
    iotas_f = pp.tile([1, 2 * P], f32, tag="iotas_f")
    nc.vector.tensor_copy(iotas_f[:, :P], iota_nf[:])
    nc.vector.tensor_copy(iotas_f[:, P:], iota_cf[:])
    ncol = pp.tile([P, 2], f32, tag="ncol")
    for a in range(2):
        nc_ps = tp_ps.tile([P, 1], f32, tag="tp")
        nc.tensor.transpose(out=nc_ps[:], in_=iotas_f[:, P * a:P * (a + 1)],
                            identity=id_f[:1, :1])
        nc.vector.tensor_copy(ncol[:, a:a + 1], nc_ps[:])
    m1 = pp.tile([P, SPC], f32, tag="m1")
    nc.vector.tensor_tensor(out=m1[:], in0=ncol[:, 0:1].to_broadcast([P, SPC]),
                            in1=ns_exp[:], op=mybir.AluOpType.is_lt)
    m2 = pp.tile([P, SPC], f32, tag="m2")
    nc.vector.tensor_tensor(out=m2[:], in0=ncol[:, 1:2].to_broadcast([P, SPC]),
                            in1=cl_exp[:], op=mybir.AluOpType.is_lt)
    mask_bag = cp.tile([P, SPC], f32, tag="mask_bag")
    nc.vector.tensor_mul(mask_bag[:], m1[:], m2[:])

    # ---------------- main loop ----------------
    for bt in range(NBT):
        comb_all = combp.tile([P, P], bf16, tag="comb")
        ctxT = ctx_ps.tile([P, 512], f32, tag="ctx")
        for j in range(32):
            tt = 32 * bt + j           # global tile index
            s = tt // 32               # sample index (== bt)
            icol = P * (tt % 2) + tt // 2
            bags = gp.tile([P, D], bf16, tag="bags")
            nc.gpsimd.indirect_dma_start(
                out=bags[:], out_offset=None, in_=emb[:],
                in_offset=bass.IndirectOffsetOnAxis(ap=idx_i[:, icol:icol + 1], axis=0))

            pre = pre_ps.tile([P, DK], f32, tag="pre")
            for ch in range(4):
                t_ps = tp_ps.tile([P, P], bf16, tag="tp")
                nc.tensor.transpose(out=t_ps[:], in_=bags[:, P * ch:P * (ch + 1)],
                                    identity=id_bf[:])
                bT = btp.tile([P, P], bf16, tag="bT")
                if ch == 3:
                    nc.scalar.copy(bT[:], t_ps[:])
                else:
                    nc.vector.tensor_copy(bT[:], t_ps[:])
                nc.tensor.matmul(out=pre[:], lhsT=bT[:], rhs=wpre_b[:, DK * ch:DK * (ch + 1)],
                                 start=(ch == 0), stop=False)
            nc.tensor.matmul(out=pre[:], lhsT=ones_bf[:],
                             rhs=q_bf[0:1, DK * s:DK * (s + 1)],
                             start=False, stop=True)

            tanh = ep.tile([P, DK], f32, tag="tanh")
            nc.scalar.activation(tanh[:], pre[:], mybir.ActivationFunctionType.Tanh)
            prod = ep.tile([P, DK], f32, tag="prod")
            nc.vector.tensor_mul(prod[:], tanh[:], v_tile[:])
            energy = ep.tile([P, 1], f32, tag="energy")
            nc.vector.tensor_reduce(out=energy[:], in_=prod[:],
                                    axis=mybir.AxisListType.X, op=mybir.AluOpType.add)
            expv = ep.tile([P, 1], f32, tag="expv")
            nc.scalar.activation(expv[:], energy[:], mybir.ActivationFunctionType.Exp)
            nc.vector.tensor_tensor(out=comb_all[:, 4 * j:4 * j + 4],
                                    in0=expv[:, :1].to_broadcast([P, 4]),
                                    in1=bm[:], op=mybir.AluOpType.mult)
            for ch in range(4):
                nc.tensor.matmul(out=ctxT[:, P * ch + 4 * j:P * ch + 4 * j + 4],
                                 lhsT=bags[:, P * ch:P * (ch + 1)],
                                 rhs=comb_all[:, 4 * j:4 * j + 4],
                                 start=True, stop=True)

        # ---- bagtile epilogue (128 bags) ----
        sums = sum_ps.tile([P, 1], f32, tag="sums")
        nc.tensor.matmul(out=sums[:], lhsT=comb_all[:], rhs=onescol_bf[:],
                         start=True, stop=True)
        recip = op.tile([P, 1], f32, tag="recip")
        nc.vector.reciprocal(recip[:], sums[:])
        scale = op.tile([P, 1], f32, tag="scale")
        nc.vector.tensor_mul(scale[:], recip[:], mask_bag[:, bt:bt + 1])
        ctx_sb = op.tile([P, 512], f32, tag="ctx_sb")
        nc.vector.tensor_copy(ctx_sb[:], ctxT[:])
        out_sb = op.tile([P, D], f32, tag="out_sb")
        for ch in range(4):
            ct_ps = tp_ps.tile([P, P], f32, tag="tp")
            nc.tensor.transpose(out=ct_ps[:], in_=ctx_sb[:, P * ch:P * (ch + 1)],
                                identity=id_f[:])
            nc.vector.tensor_scalar_mul(out_sb[:, P * ch:P * (ch + 1)], ct_ps[:],
                                        scale[:, :1])
        nc.sync.dma_start(out[P * bt:P * (bt + 1), :], out_sb[:])


_PROG = None


def _build():
    global _PROG
    if _PROG is not None:
        return _PROG
    nc = bacc.Bacc("TRN2", target_bir_lowering=False, debug=False)
    t = dict(
        ids=nc.dram_tensor("ids", [BAGS, W], i32, kind="ExternalInput").ap(),
        nlen=nc.dram_tensor("nlen", [BAGS], i32, kind="ExternalInput").ap(),
        hid=nc.dram_tensor("hid", [SPC, D], f32, kind="ExternalInput").ap(),
        emb=nc.dram_tensor("emb", [V, D], f32, kind="ExternalInput").ap(),
        wpre=nc.dram_tensor("wpre", [D, DK], f32, kind="ExternalInput").ap(),
        wq=nc.dram_tensor("wq", [D, DK], f32, kind="ExternalInput").ap(),
        bpre=nc.dram_tensor("bpre", [DK], f32, kind="ExternalInput").ap(),
        vvec=nc.dram_tensor("vvec", [DK], f32, kind="ExternalInput").ap(),
        nsz=nc.dram_tensor("nsz", [SPC * C], i32, kind="ExternalInput").ap(),
        clen=nc.dram_tensor("clen", [SPC], i32, kind="ExternalInput").ap(),
        out=nc.dram_tensor("out", [BAGS, D], f32, kind="ExternalOutput").ap(),
    )
    with tile.TileContext(nc) as tc, ExitStack() as ctx:
        _emit(nc, tc, ctx, t)
    nc.compile()
    _PROG = nc
    return nc


def _make_in_maps(inputs):
    return _shard(
        inputs["token_ids"], inputs["node_lengths"], inputs["node_sizes"],
        inputs["cross_lengths"], inputs["con_hidden"], inputs["emb"],
        inputs["W_pre"], inputs["b_pre"], inputs["W_q"], inputs["v"])


def _shard(token_ids, node_lengths, node_sizes, cross_lengths, con_hidden,
           emb, W_pre, b_pre, W_q, v):
    token_ids = np.asarray(token_ids).astype(np.int32).reshape(B * C * N, W)
    node_lengths = np.asarray(node_lengths).astype(np.int32).reshape(B * C * N)
    node_sizes = np.asarray(node_sizes).astype(np.int32).reshape(B * C)
    cross_lengths = np.asarray(cross_lengths).astype(np.int32).reshape(B)
    con_hidden = np.asarray(con_hidden).astype(np.float32).reshape(B, D)
    emb_np = np.ascontiguousarray(np.asarray(emb).astype(np.float32))
    W_pre_np = np.ascontiguousarray(np.asarray(W_pre).astype(np.float32))
    W_q_np = np.ascontiguousarray(np.asarray(W_q).astype(np.float32))
    b_pre_np = np.asarray(b_pre).astype(np.float32)
    v_np = np.asarray(v).astype(np.float32)

    in_maps = []
    for c in range(NCORES):
        bs, be = c * BAGS, (c + 1) * BAGS
        ss, se = c * SPC, (c + 1) * SPC
        in_maps.append(dict(
            ids=np.ascontiguousarray(token_ids[bs:be]),
            nlen=np.ascontiguousarray(node_lengths[bs:be]),
            hid=np.ascontiguousarray(con_hidden[ss:se]),
            emb=emb_np,
            wpre=W_pre_np,
            wq=W_q_np,
            bpre=b_pre_np,
            vvec=v_np,
            nsz=np.ascontiguousarray(node_sizes[ss * C:se * C]),
            clen=np.ascontiguousarray(cross_lengths[ss:se]),
        ))
    return in_maps


def kernel(token_ids, node_lengths, node_sizes, cross_lengths, con_hidden,
           emb, W_pre, b_pre, W_q, v):
    in_maps = _shard(token_ids, node_lengths, node_sizes, cross_lengths,
                     con_hidden, emb, W_pre, b_pre, W_q, v)
    nc = _build()
    res = run_bass_kernel_spmd(nc, in_maps, core_ids=list(range(NCORES)))
    out = np.concatenate([res.results[c]["out"] for c in range(NCORES)], axis=0)
    out = out.reshape(B, C, N, D)
    hidden = np.asarray(con_hidden).astype(np.float32).reshape(B, D).copy()
    return out, hidden
